# revision 33
# baseline (speedup 1.0000x reference)
"""Trainium2 Bass kernel for nn_GTCNN (product-graph GTCNN, 2 layers, K collapsed).

Math (per batch b, x: [M=8192, 32]):
  Adj = s0*I + s1*kron(I_t, As) + s2*kron(At, I_s) + s3*kron(At, As),  T=64, N=128
  h0 = x @ W1 + b1
  h_{l+1} = tanh((Adj @ h_l) @ Heff_l),   Heff_l = sum_k H[l, k]   (einsum collapses k)
  out = h2 @ W2 + b2

Device algorithm (the feature mix commutes with the node mixes):
  layer 1 "H-first", Heff0 folded into W1 on the host:
    w0 = x @ (W1 Heff0) + b1 Heff0
    z1 = tanh(P w0 + Q At w0)          P = s0*I + s1*As, Q = s2*I + s3*As
  layer 2 "H-last", quarter only:
    v  = P z1 + Q At z1                (t-quarter rows)
    out = tanh(v Heff1) @ W2 + b2
  (At, As symmetric -> they serve directly as matmul stationaries.)

Sharding: core c -> (b = c // 4, t-quarter q = c % 4). Layer 1 computed fully per
b (4x redundant, no collectives); layer 2 + output restricted to the 16-t quarter.

Layouts (n = 32*nh + nl, t = 32*c + tl, partition-block always nh):
  FD   [32*nh + h,  (c, tl, nl)]    feature-on-partition (W1/Heff/W2 matmuls)
  NM   [n, (c, tl, h)]              node-on-partition, t-major (P/Q "w" side)
  NM'  [n, (c, h, tl)]              node-on-partition, h-major (P/Q result, z1)
  FDT  [32*nh + tl, (c, h-or-nl, ...)]  t-on-partition (At matmuls)
All layout moves are DVE 32x32 StreamTranspose with CONTIGUOUS writes (strided
writes measured 4.6 ns/elem vs 1.17 contiguous); the unavoidable strided side
is always a read (1.8x) or a strided-inner matmul moving operand.

Perf design:
  - fp16 on-chip (1 cycle/col matmuls, fast weight load); PSUM stays fp32;
    rel err ~1e-3, tolerance is 2e-2
  - every PSUM->SBUF crossing is one ACT op (bias / copy / tanh) doing the
    fp32->fp16 conversion
  - x pre-marshalled on host into FD (one contiguous DMA); out stored FD
  - 8 warmup matmuls + dummy matmuls with data deps on mid-pipeline tiles
    keep the PE busy so the HAM clock gate never re-throttles to 1.2 GHz;
    dummies write (start=stop=True) into PSUM regions whose real writers
    also open with start=True, so they are overwritten harmlessly
"""

import numpy as np

T, NS, B, FIN, HID, FOUT = 64, 128, 2, 32, 32, 16
M = T * NS
NCORES, NQ = 8, 4
TQ = T // NQ  # 16 t's per quarter

_CACHE = {}

# column-block offsets (x128) inside the packed const tensor
_W1, _P, _Q, _H1, _W2 = 0, 1, 2, 3, 4
_ATBD = 5  # 4 blocks: 2*c + cp
_ATBQ = 9  # 2 blocks: c
_NCBLK = 11


def _build_nc():
    from contextlib import ExitStack

    import concourse.mybir as mybir
    import concourse.tile as tile
    from concourse import bacc
    from concourse.bass import ds

    fp = mybir.dt.float32
    f16 = mybir.dt.float16
    AF = mybir.ActivationFunctionType

    nc = bacc.Bacc(
        "TRN2",
        target_bir_lowering=False,
        debug=False,
        enable_asserts=False,
        num_devices=NCORES,
    )

    xfd_d = nc.dram_tensor("xfd", [128, 2048], f16, kind="ExternalInput")
    cst_d = nc.dram_tensor("cst", [128, _NCBLK * 128], f16, kind="ExternalInput")
    bias_d = nc.dram_tensor("bias", [128, 2], fp, kind="ExternalInput")
    outfd_d = nc.dram_tensor("outfd", [128, 512], fp, kind="ExternalOutput")

    C512 = [slice(512 * j, 512 * (j + 1)) for j in range(4)]
    H1024 = [slice(1024 * j, 1024 * (j + 1)) for j in range(2)]

    with tile.TileContext(nc) as tc, ExitStack() as ctx:
        const = ctx.enter_context(tc.tile_pool(name="const", bufs=1))
        st = ctx.enter_context(tc.tile_pool(name="st", bufs=1))
        ps = ctx.enter_context(tc.tile_pool(name="ps", bufs=2, space="PSUM"))

        pid = nc.tensor.partition_id()  # on PE: consumed by the vpre matmul AP
        cq = (pid % NQ) // 2  # which t-half holds this core's quarter
        tl0 = (pid % 2) * TQ  # tl offset inside that half

        # ---- PE warmup scratch (vector memset: fast, DVE idle this early) ----
        warm_sb = st.tile([128, 512], f16, tag="warm_sb")
        nc.vector.memset(warm_sb[:], 0.0)
        # Preload the tanh activation table off the critical path: without
        # this the first real ACTIVATE pays a lazy 1.3us ACT_TABLE_LOAD.
        tblw = st.tile([128, 1], fp, tag="tblw")
        nc.scalar.activation(tblw[:], warm_sb[:, 0:1], AF.Tanh)

        # ---- loads, chunked so w0pre's first matmuls start before the full
        # transfers land: W1'/P/Q first, then x in halves, then the rest ----
        cs = const.tile([128, _NCBLK * 128], f16, tag="cs")
        x_fd = st.tile([128, 2048], f16, tag="x_fd")
        bt = const.tile([128, 2], fp, tag="bt")
        nc.sync.dma_start(cs[:, 0:384], cst_d.ap()[:, 0:384])
        for j in range(4):
            nc.sync.dma_start(x_fd[:, C512[j]], xfd_d.ap()[:, C512[j]])
        nc.sync.dma_start(cs[:, 384:], cst_d.ap()[:, 384:])
        nc.sync.dma_start(bt[:], bias_d.ap())

        def blk(i):
            return cs[:, 128 * i : 128 * (i + 1)]

        w1m, pmat, qmat, h1m, w2m = blk(_W1), blk(_P), blk(_Q), blk(_H1), blk(_W2)
        b1t = bt[:, 0:1]
        b2t = bt[:, 1:2]
        mm = nc.tensor.matmul

        def dummy(out, src):
            """Keep-warm matmul: garbage into a region whose real writer
            opens with start=True. Fires when `src` (SBUF fp16) is ready."""
            mm(out, src[:, 0:128], src[:, 0:512], start=True, stop=True,
               skip_group_check=True)

        # PSUM tiles are split in HALVES (pA/pB): consumers of a PSUM tile
        # wait for its ENTIRE matmul group (no subtile deps on PSUM), so
        # smaller tiles let ACT start at half-group completion.
        # ---- PE warmup: a few matmuls start the HAM clock ramp while the
        # DMAs land. They write u0's buffer, so w0pre has no WAW dependence
        # on them and starts as soon as the DMA semaphores are visible.
        w0a = ps.tile([128, 1024], fp, tag="pA")
        w0b = ps.tile([128, 1024], fp, tag="pB")
        u0a = ps.tile([128, 1024], fp, tag="pA")
        u0b = ps.tile([128, 1024], fp, tag="pB")
        for _ in range(3):
            dummy(u0a[:, 0:512], warm_sb)

        # =========================== layer 1 (full) ===========================
        # w0 = x @ W1' + b1'   -> FD [h-part, (c, tl, nl)]
        w0h = [w0a, w0a, w0b, w0b]
        for j in range(4):
            mm(w0h[j][:, 512 * (j % 2) : 512 * (j % 2 + 1)], w1m, x_fd[:, C512[j]],
               start=True, stop=True)
        w0_fd = st.tile([128, 2048], f16, tag="w0_fd")
        for j in range(4):
            nc.scalar.activation(
                w0_fd[:, C512[j]], w0h[j][:, 512 * (j % 2) : 512 * (j % 2 + 1)],
                AF.Identity, bias=b1t)

        # g0 = FDT of w0 [tl-part, (c, nl, h)]: strided read, contiguous write
        g0 = st.tile([128, 2048], f16, tag="g0")
        gi = w0_fd[:].rearrange("p (c tl nl) -> p c nl tl", c=2, tl=32, nl=32)
        go = g0[:].rearrange("p (c nl h) -> p c nl h", c=2, nl=32, h=32)
        dummy(u0a[:, 0:512], w0_fd)  # fires mid ACT/transpose phase
        for c in range(2):
            nc.vector.transpose(out=go[:, c], in_=gi[:, c])

        # u0 = At-mix(w0): FDT, PSUM-accum over c -> free (cp, nl, h).
        # Moving operands are plain contiguous 512-slices of g0 (nl-halves):
        # strided/3D moving APs measured 4 cycles/col vs 1 contiguous.
        # c outermost: all 4 start-matmuls depend only on g0's first t-half,
        # so they overlap the transpose of the second half.
        dummy(u0a[:, 0:512], g0)
        u0h = [u0a, u0b]
        for c in range(2):
            for cp in range(2):
                for nn in range(2):
                    mm(
                        u0h[cp][:, 512 * nn : 512 * (nn + 1)],
                        blk(_ATBD + 2 * c + cp),
                        g0[:, c * 1024 + 512 * nn : c * 1024 + 512 * (nn + 1)],
                        start=(c == 0),
                        stop=(c == 1),
                    )
        u0_sb = st.tile([128, 2048], f16, tag="u0_sb")
        for j in range(2):
            nc.scalar.activation(u0_sb[:, H1024[j]], u0h[j][:], AF.Identity)

        # u0_nm [n, (cp, h, tl')]: strided read, contiguous write, 4-way
        # chunked (h-halves) so each chunk feeds its Q-matmul immediately
        u0_nm = st.tile([128, 2048], f16, tag="u0_nm")
        uiv = u0_sb[:].rearrange("p (cp nl h) -> p cp h nl", cp=2, nl=32, h=32)
        for cp in range(2):
            for hh in range(2):
                nc.vector.transpose(
                    out=u0_nm[:, cp * 1024 + 512 * hh : cp * 1024 + 512 * (hh + 1)],
                    in_=uiv[:, cp, 16 * hh : 16 * (hh + 1), :],
                )

        # w0_nm [n, (c, tl, h)], contiguous both sides. Emitted late so the
        # DVE static scheduler keeps the critical g0/u0_nm chunks together
        # (w0_nm only feeds the P-matmuls, which hide under the u0 path).
        w0_nm = st.tile([128, 2048], f16, tag="w0_nm")
        for j in range(2):
            nc.vector.transpose(out=w0_nm[:, H1024[j]], in_=w0_fd[:, H1024[j]])

        # z1 = tanh(P w0 + Q u0)  -> NM' [n, (c, h, tl)]
        # P moving: w0_nm viewed (c, h, tl) = strided-inner; Q moving: contiguous
        za = ps.tile([128, 1024], fp, tag="pA")
        zb = ps.tile([128, 1024], fp, tag="pB")
        zh = [za, za, zb, zb]
        dummy(za[:, 0:512], u0_sb)
        w0v = w0_nm[:].rearrange("p (c tl h) -> p c h tl", c=2, tl=32, h=32)
        for c in range(2):
            for hh in range(2):
                j = 2 * c + hh
                mm(zh[j][:, 512 * (j % 2) : 512 * (j % 2 + 1)], pmat,
                   w0v[:, c, 16 * hh : 16 * (hh + 1), :], start=True, stop=False)
        for j in range(4):
            mm(zh[j][:, 512 * (j % 2) : 512 * (j % 2 + 1)], qmat,
               u0_nm[:, C512[j]], start=False, stop=True)
        z1_nm = st.tile([128, 2048], f16, tag="z1_nm")
        for j in range(4):
            nc.scalar.activation(
                z1_nm[:, C512[j]], zh[j][:, 512 * (j % 2) : 512 * (j % 2 + 1)], AF.Tanh)

        # ====================== layer 2 (t-quarter only) ======================
        # g1 = FDT of z1 [tl-part, (c, h, nl)]: contiguous both sides
        g1 = st.tile([128, 2048], f16, tag="g1")
        u1_ps = ps.tile([128, 1024], fp, tag="pA")
        dummy(u1_ps[:, 0:512], z1_nm)
        for j in range(4):
            nc.vector.transpose(out=g1[:, C512[j]], in_=z1_nm[:, C512[j]])

        # vpre's P-half only needs z1, so it is emitted before the u1 group
        # and fills the PE while the g1 transposes run
        vpre = ps.tile([128, 512], fp, tag="pB")
        z1v = z1_nm[:].rearrange("p (c h tl) -> p c tl h", c=2, h=32, tl=32)
        mm(vpre[:], pmat, z1v[:, ds(cq, 1), ds(tl0, TQ), :], start=True, stop=False)

        # u1 = At[quarter,:]-mix(z1): PSUM-accum over c -> free (h, nl), part (nh, tq)
        g1r = g1[:].rearrange("p (c h nl) -> p c h nl", c=2, h=32, nl=32)
        for c in range(2):
            for hh in range(2):
                mm(
                    u1_ps[:, 512 * hh : 512 * (hh + 1)],
                    blk(_ATBQ + c),
                    g1r[:, c, 16 * hh : 16 * (hh + 1), :],
                    start=(c == 0),
                    stop=(c == 1),
                )
        u1_sb = st.tile([128, 1024], f16, tag="u1_sb")
        nc.scalar.activation(u1_sb[:], u1_ps[:], AF.Identity)

        # u1_nm [n, (h, tq32)]: contiguous both sides
        u1_nm = st.tile([128, 1024], f16, tag="u1_nm")
        nc.vector.transpose(out=u1_nm[:], in_=u1_sb[:])

        # v = P z1[quarter] + Q u1  -> NM quarter, free (tq, h)
        u1v = u1_nm[:].rearrange("p (h t) -> p t h", h=32, t=32)
        mm(vpre[:], qmat, u1v[:, 0:TQ, :], start=False, stop=True)

        # tail: everything 2-way chunked (tq halves) so the six remaining
        # serial 512-wide ops overlap pairwise across ACT/DVE/PE
        Q256 = [slice(256 * j, 256 * (j + 1)) for j in range(2)]
        v_sb = st.tile([128, 512], f16, tag="v_sb")
        for j in range(2):
            nc.scalar.activation(v_sb[:, Q256[j]], vpre[:, Q256[j]], AF.Identity)

        # v_fd [h-part, (tq, nl)]: contiguous both sides
        v_fd = st.tile([128, 512], f16, tag="v_fd")
        h2pre = ps.tile([128, 512], fp, tag="pA")
        dummy(h2pre[:], v_sb)
        for j in range(2):
            nc.vector.transpose(out=v_fd[:, Q256[j]], in_=v_sb[:, Q256[j]])

        # z2 = tanh(v @ H1')  (FD); out = z2 @ W2' + b2  (FD)
        z2_fd = st.tile([128, 512], f16, tag="z2_fd")
        for j in range(2):
            mm(h2pre[:, Q256[j]], h1m, v_fd[:, Q256[j]], start=True, stop=True)
        for j in range(2):
            nc.scalar.activation(z2_fd[:, Q256[j]], h2pre[:, Q256[j]], AF.Tanh)

        opre = ps.tile([128, 512], fp, tag="pB")
        out_fd = st.tile([128, 512], fp, tag="out_fd")
        for j in range(2):
            mm(opre[:, Q256[j]], w2m, z2_fd[:, Q256[j]], start=True, stop=True)
        for j in range(2):
            nc.scalar.activation(out_fd[:, Q256[j]], opre[:, Q256[j]], AF.Identity, bias=b2t)

        # store in FD layout; the host unscrambles
        nc.sync.dma_start(outfd_d.ap(), out_fd[:])

    nc.compile()
    return nc


def _host_weights(Adj_t, Adj_s, s, H, W1, b1, W2, b2):
    f4 = np.float32
    I4 = np.eye(4, dtype=f4)
    I128 = np.eye(128, dtype=f4)
    Heff = H.sum(axis=1).astype(f4)  # [2, 32, 32]

    P = (s[0] * I128 + s[1] * Adj_s).astype(f4)
    Q = (s[2] * I128 + s[3] * Adj_s).astype(f4)

    W1p = (W1 @ Heff[0]).astype(f4)  # H-first: fold Heff0 into W1
    b1p = (b1 @ Heff[0]).astype(f4)
    w2pad = np.zeros((32, 32), dtype=f4)
    w2pad[:, :FOUT] = W2

    cst = np.zeros((NQ, 128, _NCBLK * 128), dtype=np.float16)
    for q in range(NQ):
        c = cst[q]
        c[:, 0:128] = np.kron(I4, W1p)
        c[:, 128:256] = P
        c[:, 256:384] = Q
        c[:, 384:512] = np.kron(I4, Heff[1])
        c[:, 512:640] = np.kron(I4, w2pad)
        for cc in range(2):
            for cp in range(2):
                i = _ATBD + 2 * cc + cp
                c[:, 128 * i : 128 * (i + 1)] = np.kron(
                    I4, Adj_t[32 * cc : 32 * (cc + 1), 32 * cp : 32 * (cp + 1)].astype(f4)
                )
        for cc in range(2):
            bq = np.zeros((32, 32), dtype=f4)
            bq[:, :TQ] = Adj_t[32 * cc : 32 * (cc + 1), TQ * q : TQ * (q + 1)]
            i = _ATBQ + cc
            c[:, 128 * i : 128 * (i + 1)] = np.kron(I4, bq)

    bias = np.zeros((128, 2), dtype=f4)
    bias[:, 0] = np.tile(b1p, 4)
    b2pad = np.zeros(32, dtype=f4)
    b2pad[:FOUT] = b2
    bias[:, 1] = np.tile(b2pad, 4)
    return cst, bias


def _in_maps(inputs):
    f4 = np.float32
    x = np.asarray(inputs["x"], dtype=f4)
    cst, bias = _host_weights(
        np.asarray(inputs["Adj_t"], dtype=f4),
        np.asarray(inputs["Adj_s"], dtype=f4),
        np.asarray(inputs["s"], dtype=f4),
        np.asarray(inputs["H"], dtype=f4),
        np.asarray(inputs["W1"], dtype=f4),
        np.asarray(inputs["b1"], dtype=f4),
        np.asarray(inputs["W2"], dtype=f4),
        np.asarray(inputs["b2"], dtype=f4),
    )
    # FD-marshalled x per batch: xfd[32*nh + f, 32*t + nl] = x[b, 128*t + 32*nh + nl, f]
    xfd = [
        np.ascontiguousarray(
            x[b].reshape(T, 4, 32, FIN).transpose(1, 3, 0, 2).reshape(128, 2048)
        ).astype(np.float16)
        for b in range(B)
    ]
    maps = []
    for c in range(NCORES):
        b, q = c // NQ, c % NQ
        maps.append(
            {"xfd": xfd[b], "cst": np.ascontiguousarray(cst[q]), "bias": bias}
        )
    return maps


def kernel(**inputs) -> np.ndarray:
    import os

    from concourse import bass_utils

    if "nc" not in _CACHE:
        _CACHE["nc"] = _build_nc()
    nc = _CACHE["nc"]

    maps = _in_maps(inputs)

    trace = bool(int(os.environ.get("GTCNN_TRACE", "0")))
    res = bass_utils.run_bass_kernel_spmd(
        nc,
        maps,
        core_ids=list(range(NCORES)),
        trace=trace,
        trace_cores=list(range(NCORES)) if trace else None,
        stitch_traces=False,
    )
    _CACHE["last_results"] = res

    out = np.empty((B, M, FOUT), dtype=np.float32)
    for c in range(NCORES):
        b, q = c // NQ, c % NQ
        arr = np.asarray(res.results[c]["outfd"]).reshape(4, 32, TQ, 32)
        out[b, 2048 * q : 2048 * (q + 1), :] = (
            arr[:, :FOUT, :, :].transpose(2, 0, 3, 1).reshape(2048, FOUT)
        )
    return out


# revision 35
# speedup vs baseline: 1.0658x; 1.0658x over previous
"""Trainium2 Bass kernel for nn_GTCNN (product-graph GTCNN, 2 layers, K collapsed).

Math (per batch b, x: [M=8192, 32]):
  Adj = s0*I + s1*kron(I_t, As) + s2*kron(At, I_s) + s3*kron(At, As),  T=64, N=128
  h0 = x @ W1 + b1
  h_{l+1} = tanh((Adj @ h_l) @ Heff_l),   Heff_l = sum_k H[l, k]   (einsum collapses k)
  out = h2 @ W2 + b2

Device algorithm (the feature mix commutes with the node mixes):
  layer 1 "H-first", Heff0 folded into W1 on the host:
    w0 = x @ (W1 Heff0) + b1 Heff0
    z1 = tanh(P w0 + Q At w0)          P = s0*I + s1*As, Q = s2*I + s3*As
  layer 2 "H-last", quarter only:
    v  = P z1 + Q At z1                (t-quarter rows)
    out = tanh(v Heff1) @ W2 + b2
  (At, As symmetric -> they serve directly as matmul stationaries.)

Sharding: core c -> (b = c // 4, t-quarter q = c % 4). Layer 1 computed fully per
b (4x redundant, no collectives); layer 2 + output restricted to the 16-t quarter.

Layouts (n = 32*nh + nl, t = 32*c + tl, partition-block always nh):
  FD   [32*nh + h,  (c, tl, nl)]    feature-on-partition (W1/Heff/W2 matmuls)
  NM   [n, (c, tl, h)]              node-on-partition, t-major (P/Q "w" side)
  NM'  [n, (c, h, tl)]              node-on-partition, h-major (P/Q result, z1)
  FDT  [32*nh + tl, (c, h-or-nl, ...)]  t-on-partition (At matmuls)
All layout moves are DVE 32x32 StreamTranspose with CONTIGUOUS writes (strided
writes measured 4.6 ns/elem vs 1.17 contiguous); the unavoidable strided side
is always a read (1.8x) or a strided-inner matmul moving operand.

Perf design:
  - fp16 on-chip (1 cycle/col matmuls, fast weight load); PSUM stays fp32;
    rel err ~1e-3, tolerance is 2e-2
  - every PSUM->SBUF crossing is one ACT op (bias / copy / tanh) doing the
    fp32->fp16 conversion
  - x pre-marshalled on host into FD (one contiguous DMA); out stored FD
  - 8 warmup matmuls + dummy matmuls with data deps on mid-pipeline tiles
    keep the PE busy so the HAM clock gate never re-throttles to 1.2 GHz;
    dummies write (start=stop=True) into PSUM regions whose real writers
    also open with start=True, so they are overwritten harmlessly
"""

import numpy as np

T, NS, B, FIN, HID, FOUT = 64, 128, 2, 32, 32, 16
M = T * NS
NCORES, NQ = 8, 4
TQ = T // NQ  # 16 t's per quarter

_CACHE = {}

# column-block offsets (x128) inside the packed const tensor
_W1, _P, _Q, _H1, _W2 = 0, 1, 2, 3, 4
_ATBD = 5  # 4 blocks: 2*c + cp
_ATBQ = 9  # 2 blocks: c
_NCBLK = 11


def _build_nc():
    from contextlib import ExitStack

    import concourse.mybir as mybir
    import concourse.tile as tile
    from concourse import bacc
    from concourse.bass import ds

    fp = mybir.dt.float32
    f16 = mybir.dt.float16
    AF = mybir.ActivationFunctionType

    nc = bacc.Bacc(
        "TRN2",
        target_bir_lowering=False,
        debug=False,
        enable_asserts=False,
        num_devices=NCORES,
    )

    xfd_d = nc.dram_tensor("xfd", [128, 2048], f16, kind="ExternalInput")
    cst_d = nc.dram_tensor("cst", [128, _NCBLK * 128], f16, kind="ExternalInput")
    bias_d = nc.dram_tensor("bias", [128, 2], fp, kind="ExternalInput")
    outfd_d = nc.dram_tensor("outfd", [128, 512], fp, kind="ExternalOutput")

    C512 = [slice(512 * j, 512 * (j + 1)) for j in range(4)]
    H1024 = [slice(1024 * j, 1024 * (j + 1)) for j in range(2)]

    with tile.TileContext(nc) as tc, ExitStack() as ctx:
        const = ctx.enter_context(tc.tile_pool(name="const", bufs=1))
        st = ctx.enter_context(tc.tile_pool(name="st", bufs=1))
        ps = ctx.enter_context(tc.tile_pool(name="ps", bufs=2, space="PSUM"))

        pid = nc.tensor.partition_id()  # on PE: consumed by the vpre matmul AP
        cq = (pid % NQ) // 2  # which t-half holds this core's quarter
        tl0 = (pid % 2) * TQ  # tl offset inside that half

        # ---- PE warmup scratch (vector memset: fast, DVE idle this early) ----
        warm_sb = st.tile([128, 512], f16, tag="warm_sb")
        nc.vector.memset(warm_sb[:], 0.0)
        # Preload the tanh activation table off the critical path: without
        # this the first real ACTIVATE pays a lazy 1.3us ACT_TABLE_LOAD.
        tblw = st.tile([128, 1], fp, tag="tblw")
        nc.scalar.activation(tblw[:], warm_sb[:, 0:1], AF.Tanh)

        # ---- loads, chunked so w0pre's first matmuls start before the full
        # transfers land: W1'/P/Q first, then x in halves, then the rest ----
        cs = const.tile([128, _NCBLK * 128], f16, tag="cs")
        x_fd = st.tile([128, 2048], f16, tag="x_fd")
        bt = const.tile([128, 2], fp, tag="bt")
        nc.sync.dma_start(bt[:], bias_d.ap())  # tiny; ACT-w0 needs it first
        nc.sync.dma_start(cs[:, 0:384], cst_d.ap()[:, 0:384])
        for j in range(4):
            nc.sync.dma_start(x_fd[:, C512[j]], xfd_d.ap()[:, C512[j]])
        nc.sync.dma_start(cs[:, 384:], cst_d.ap()[:, 384:])

        def blk(i):
            return cs[:, 128 * i : 128 * (i + 1)]

        w1m, pmat, qmat, h1m, w2m = blk(_W1), blk(_P), blk(_Q), blk(_H1), blk(_W2)
        b1t = bt[:, 0:1]
        b2t = bt[:, 1:2]
        mm = nc.tensor.matmul

        def dummy(out, src):
            """Keep-warm matmul: garbage into a region whose real writer
            opens with start=True. Fires when `src` (SBUF fp16) is ready."""
            mm(out, src[:, 0:128], src[:, 0:512], start=True, stop=True,
               skip_group_check=True)

        # PSUM tiles are split in HALVES (pA/pB): consumers of a PSUM tile
        # wait for its ENTIRE matmul group (no subtile deps on PSUM), so
        # smaller tiles let ACT start at half-group completion.
        # ---- PE warmup: a few matmuls start the HAM clock ramp while the
        # DMAs land. They write u0's buffer, so w0pre has no WAW dependence
        # on them and starts as soon as the DMA semaphores are visible.
        w0a = ps.tile([128, 1024], fp, tag="pA")
        w0b = ps.tile([128, 1024], fp, tag="pB")
        u0a = ps.tile([128, 1024], fp, tag="pA")
        u0b = ps.tile([128, 1024], fp, tag="pB")
        for _ in range(3):
            dummy(u0a[:, 0:512], warm_sb)

        # =========================== layer 1 (full) ===========================
        # w0 = x @ W1' + b1'   -> FD [h-part, (c, tl, nl)]
        w0h = [w0a, w0a, w0b, w0b]
        for j in range(4):
            mm(w0h[j][:, 512 * (j % 2) : 512 * (j % 2 + 1)], w1m, x_fd[:, C512[j]],
               start=True, stop=True)
        w0_fd = st.tile([128, 2048], f16, tag="w0_fd")
        for j in range(4):
            nc.scalar.activation(
                w0_fd[:, C512[j]], w0h[j][:, 512 * (j % 2) : 512 * (j % 2 + 1)],
                AF.Identity, bias=b1t)

        # g0 = FDT of w0 [tl-part, (c, nl, h)]: strided read, contiguous write
        g0 = st.tile([128, 2048], f16, tag="g0")
        gi = w0_fd[:].rearrange("p (c tl nl) -> p c nl tl", c=2, tl=32, nl=32)
        go = g0[:].rearrange("p (c nl h) -> p c nl h", c=2, nl=32, h=32)
        dummy(u0a[:, 0:512], w0_fd)  # fires mid ACT/transpose phase
        for c in range(2):
            nc.vector.transpose(out=go[:, c], in_=gi[:, c])

        # u0 = At-mix(w0): FDT, PSUM-accum over c -> free (cp, nl, h).
        # Moving operands are plain contiguous 512-slices of g0 (nl-halves):
        # strided/3D moving APs measured 4 cycles/col vs 1 contiguous.
        # c outermost: all 4 start-matmuls depend only on g0's first t-half,
        # so they overlap the transpose of the second half.
        dummy(u0a[:, 0:512], g0)
        u0h = [u0a, u0b]
        for c in range(2):
            for cp in range(2):
                for nn in range(2):
                    mm(
                        u0h[cp][:, 512 * nn : 512 * (nn + 1)],
                        blk(_ATBD + 2 * c + cp),
                        g0[:, c * 1024 + 512 * nn : c * 1024 + 512 * (nn + 1)],
                        start=(c == 0),
                        stop=(c == 1),
                    )
        u0_sb = st.tile([128, 2048], f16, tag="u0_sb")
        for j in range(2):
            nc.scalar.activation(u0_sb[:, H1024[j]], u0h[j][:], AF.Identity)

        # u0_nm [n, (cp, h, tl')]: strided read, contiguous write, 4-way
        # chunked (h-halves) so each chunk feeds its Q-matmul immediately
        u0_nm = st.tile([128, 2048], f16, tag="u0_nm")
        uiv = u0_sb[:].rearrange("p (cp nl h) -> p cp h nl", cp=2, nl=32, h=32)
        for cp in range(2):
            for hh in range(2):
                nc.vector.transpose(
                    out=u0_nm[:, cp * 1024 + 512 * hh : cp * 1024 + 512 * (hh + 1)],
                    in_=uiv[:, cp, 16 * hh : 16 * (hh + 1), :],
                )

        # w0_nm [n, (c, tl, h)], contiguous both sides. Emitted late so the
        # DVE static scheduler keeps the critical g0/u0_nm chunks together
        # (w0_nm only feeds the P-matmuls, which hide under the u0 path).
        w0_nm = st.tile([128, 2048], f16, tag="w0_nm")
        for j in range(2):
            nc.vector.transpose(out=w0_nm[:, H1024[j]], in_=w0_fd[:, H1024[j]])

        # z1 = tanh(P w0 + Q u0)  -> NM' [n, (c, h, tl)]
        # P moving: w0_nm viewed (c, h, tl) = strided-inner; Q moving: contiguous
        za = ps.tile([128, 1024], fp, tag="pA")
        zb = ps.tile([128, 1024], fp, tag="pB")
        zh = [za, za, zb, zb]
        dummy(za[:, 0:512], u0_sb)
        w0v = w0_nm[:].rearrange("p (c tl h) -> p c h tl", c=2, tl=32, h=32)
        # P/Q interleaved per chunk: the strided P-matmuls (4 cyc/col) would
        # otherwise queue as one block in front of all the Q-matmuls
        for c in range(2):
            for hh in range(2):
                j = 2 * c + hh
                reg = zh[j][:, 512 * (j % 2) : 512 * (j % 2 + 1)]
                mm(reg, pmat, w0v[:, c, 16 * hh : 16 * (hh + 1), :],
                   start=True, stop=False)
                mm(reg, qmat, u0_nm[:, C512[j]], start=False, stop=True)
        z1_nm = st.tile([128, 2048], f16, tag="z1_nm")
        for j in range(4):
            nc.scalar.activation(
                z1_nm[:, C512[j]], zh[j][:, 512 * (j % 2) : 512 * (j % 2 + 1)], AF.Tanh)

        # ====================== layer 2 (t-quarter only) ======================
        # g1 = FDT of z1 [tl-part, (c, h, nl)]: contiguous both sides
        g1 = st.tile([128, 2048], f16, tag="g1")
        u1_ps = ps.tile([128, 1024], fp, tag="pA")
        dummy(u1_ps[:, 0:512], z1_nm)
        for j in range(4):
            nc.vector.transpose(out=g1[:, C512[j]], in_=z1_nm[:, C512[j]])

        # vpre's P-half only needs z1, so it is emitted before the u1 group
        # and fills the PE while the g1 transposes run
        vpre = ps.tile([128, 512], fp, tag="pB")
        z1v = z1_nm[:].rearrange("p (c h tl) -> p c tl h", c=2, h=32, tl=32)
        mm(vpre[:], pmat, z1v[:, ds(cq, 1), ds(tl0, TQ), :], start=True, stop=False)

        # u1 = At[quarter,:]-mix(z1): PSUM-accum over c -> free (h, nl), part (nh, tq)
        g1r = g1[:].rearrange("p (c h nl) -> p c h nl", c=2, h=32, nl=32)
        for c in range(2):
            for hh in range(2):
                mm(
                    u1_ps[:, 512 * hh : 512 * (hh + 1)],
                    blk(_ATBQ + c),
                    g1r[:, c, 16 * hh : 16 * (hh + 1), :],
                    start=(c == 0),
                    stop=(c == 1),
                )
        u1_sb = st.tile([128, 1024], f16, tag="u1_sb")
        nc.scalar.activation(u1_sb[:], u1_ps[:], AF.Identity)

        # u1_nm [n, (h, tq32)]: contiguous both sides
        u1_nm = st.tile([128, 1024], f16, tag="u1_nm")
        nc.vector.transpose(out=u1_nm[:], in_=u1_sb[:])

        # v = P z1[quarter] + Q u1  -> NM quarter, free (tq, h)
        u1v = u1_nm[:].rearrange("p (h t) -> p t h", h=32, t=32)
        mm(vpre[:], qmat, u1v[:, 0:TQ, :], start=False, stop=True)

        # tail: everything 2-way chunked (tq halves) so the six remaining
        # serial 512-wide ops overlap pairwise across ACT/DVE/PE
        Q256 = [slice(256 * j, 256 * (j + 1)) for j in range(2)]
        v_sb = st.tile([128, 512], f16, tag="v_sb")
        for j in range(2):
            nc.scalar.activation(v_sb[:, Q256[j]], vpre[:, Q256[j]], AF.Identity)

        # v_fd [h-part, (tq, nl)]: contiguous both sides
        v_fd = st.tile([128, 512], f16, tag="v_fd")
        h2pre = ps.tile([128, 512], fp, tag="pA")
        dummy(h2pre[:], v_sb)
        for j in range(2):
            nc.vector.transpose(out=v_fd[:, Q256[j]], in_=v_sb[:, Q256[j]])

        # z2 = tanh(v @ H1')  (FD); out = z2 @ W2' + b2  (FD)
        z2_fd = st.tile([128, 512], f16, tag="z2_fd")
        for j in range(2):
            mm(h2pre[:, Q256[j]], h1m, v_fd[:, Q256[j]], start=True, stop=True)
        for j in range(2):
            nc.scalar.activation(z2_fd[:, Q256[j]], h2pre[:, Q256[j]], AF.Tanh)

        opre = ps.tile([128, 512], fp, tag="pB")
        out_fd = st.tile([128, 512], fp, tag="out_fd")
        for j in range(2):
            mm(opre[:, Q256[j]], w2m, z2_fd[:, Q256[j]], start=True, stop=True)
        for j in range(2):
            nc.scalar.activation(out_fd[:, Q256[j]], opre[:, Q256[j]], AF.Identity, bias=b2t)

        # store in FD layout; the host unscrambles
        nc.sync.dma_start(outfd_d.ap(), out_fd[:])

    nc.compile()
    return nc


def _host_weights(Adj_t, Adj_s, s, H, W1, b1, W2, b2):
    f4 = np.float32
    I4 = np.eye(4, dtype=f4)
    I128 = np.eye(128, dtype=f4)
    Heff = H.sum(axis=1).astype(f4)  # [2, 32, 32]

    P = (s[0] * I128 + s[1] * Adj_s).astype(f4)
    Q = (s[2] * I128 + s[3] * Adj_s).astype(f4)

    W1p = (W1 @ Heff[0]).astype(f4)  # H-first: fold Heff0 into W1
    b1p = (b1 @ Heff[0]).astype(f4)
    w2pad = np.zeros((32, 32), dtype=f4)
    w2pad[:, :FOUT] = W2

    cst = np.zeros((NQ, 128, _NCBLK * 128), dtype=np.float16)
    for q in range(NQ):
        c = cst[q]
        c[:, 0:128] = np.kron(I4, W1p)
        c[:, 128:256] = P
        c[:, 256:384] = Q
        c[:, 384:512] = np.kron(I4, Heff[1])
        c[:, 512:640] = np.kron(I4, w2pad)
        for cc in range(2):
            for cp in range(2):
                i = _ATBD + 2 * cc + cp
                c[:, 128 * i : 128 * (i + 1)] = np.kron(
                    I4, Adj_t[32 * cc : 32 * (cc + 1), 32 * cp : 32 * (cp + 1)].astype(f4)
                )
        for cc in range(2):
            bq = np.zeros((32, 32), dtype=f4)
            bq[:, :TQ] = Adj_t[32 * cc : 32 * (cc + 1), TQ * q : TQ * (q + 1)]
            i = _ATBQ + cc
            c[:, 128 * i : 128 * (i + 1)] = np.kron(I4, bq)

    bias = np.zeros((128, 2), dtype=f4)
    bias[:, 0] = np.tile(b1p, 4)
    b2pad = np.zeros(32, dtype=f4)
    b2pad[:FOUT] = b2
    bias[:, 1] = np.tile(b2pad, 4)
    return cst, bias


def _in_maps(inputs):
    f4 = np.float32
    x = np.asarray(inputs["x"], dtype=f4)
    cst, bias = _host_weights(
        np.asarray(inputs["Adj_t"], dtype=f4),
        np.asarray(inputs["Adj_s"], dtype=f4),
        np.asarray(inputs["s"], dtype=f4),
        np.asarray(inputs["H"], dtype=f4),
        np.asarray(inputs["W1"], dtype=f4),
        np.asarray(inputs["b1"], dtype=f4),
        np.asarray(inputs["W2"], dtype=f4),
        np.asarray(inputs["b2"], dtype=f4),
    )
    # FD-marshalled x per batch: xfd[32*nh + f, 32*t + nl] = x[b, 128*t + 32*nh + nl, f]
    xfd = [
        np.ascontiguousarray(
            x[b].reshape(T, 4, 32, FIN).transpose(1, 3, 0, 2).reshape(128, 2048)
        ).astype(np.float16)
        for b in range(B)
    ]
    maps = []
    for c in range(NCORES):
        b, q = c // NQ, c % NQ
        maps.append(
            {"xfd": xfd[b], "cst": np.ascontiguousarray(cst[q]), "bias": bias}
        )
    return maps


def kernel(**inputs) -> np.ndarray:
    import os

    from concourse import bass_utils

    if "nc" not in _CACHE:
        _CACHE["nc"] = _build_nc()
    nc = _CACHE["nc"]

    maps = _in_maps(inputs)

    trace = bool(int(os.environ.get("GTCNN_TRACE", "0")))
    res = bass_utils.run_bass_kernel_spmd(
        nc,
        maps,
        core_ids=list(range(NCORES)),
        trace=trace,
        trace_cores=list(range(NCORES)) if trace else None,
        stitch_traces=False,
    )
    _CACHE["last_results"] = res

    out = np.empty((B, M, FOUT), dtype=np.float32)
    for c in range(NCORES):
        b, q = c // NQ, c % NQ
        arr = np.asarray(res.results[c]["outfd"]).reshape(4, 32, TQ, 32)
        out[b, 2048 * q : 2048 * (q + 1), :] = (
            arr[:, :FOUT, :, :].transpose(2, 0, 3, 1).reshape(2048, FOUT)
        )
    return out


# revision 41
# speedup vs baseline: 1.3264x; 1.2445x over previous
"""Trainium2 Bass kernel for nn_GTCNN (product-graph GTCNN, 2 layers, K collapsed).

Math (per batch b, x: [M=8192, 32]):
  Adj = s0*I + s1*kron(I_t, As) + s2*kron(At, I_s) + s3*kron(At, As),  T=64, N=128
  h0 = x @ W1 + b1
  h_{l+1} = tanh((Adj @ h_l) @ Heff_l),   Heff_l = sum_k H[l, k]   (einsum collapses k)
  out = h2 @ W2 + b2

Device algorithm (the feature mix commutes with the node mixes):
  layer 1 "H-first", Heff0 folded into W1 on the host:
    w0 = x @ (W1 Heff0) + b1 Heff0
    z1 = tanh(P w0 + Q At w0)          P = s0*I + s1*As, Q = s2*I + s3*As
  layer 2 "H-last", quarter only:
    v  = P z1 + Q At z1                (t-quarter rows)
    out = tanh(v Heff1) @ W2 + b2
  (At, As symmetric -> they serve directly as matmul stationaries.)

Sharding: core c -> (b = c // 4, t-quarter q = c % 4). Layer 1 computed fully per
b (4x redundant, no collectives); layer 2 + output restricted to the 16-t quarter.

Layouts (n = 32*nh + nl, t = 32*c + tl, partition-block always nh):
  FD   [32*nh + h,  (c, tl, nl)]    feature-on-partition (W1/Heff/W2 matmuls)
  NM   [n, (c, tl, h)]              node-on-partition, t-major (P/Q "w" side)
  NM'  [n, (c, h, tl)]              node-on-partition, h-major (P/Q result, z1)
  FDT  [32*nh + tl, (c, h-or-nl, ...)]  t-on-partition (At matmuls)
All layout moves are DVE 32x32 StreamTranspose with CONTIGUOUS writes (strided
writes measured 4.6 ns/elem vs 1.17 contiguous); the unavoidable strided side
is always a read (1.8x) or a strided-inner matmul moving operand.

Perf design:
  - fp16 on-chip (1 cycle/col matmuls, fast weight load); PSUM stays fp32;
    rel err ~1e-3, tolerance is 2e-2
  - every PSUM->SBUF crossing is one ACT op (bias / copy / tanh) doing the
    fp32->fp16 conversion
  - x pre-marshalled on host into FD (one contiguous DMA); out stored FD
  - 8 warmup matmuls + dummy matmuls with data deps on mid-pipeline tiles
    keep the PE busy so the HAM clock gate never re-throttles to 1.2 GHz;
    dummies write (start=stop=True) into PSUM regions whose real writers
    also open with start=True, so they are overwritten harmlessly
"""

import numpy as np

T, NS, B, FIN, HID, FOUT = 64, 128, 2, 32, 32, 16
M = T * NS
NCORES, NQ = 8, 4
TQ = T // NQ  # 16 t's per quarter

_CACHE = {}

# column-block offsets (x128) inside the packed const tensor
_W1, _P, _Q, _H1, _W2 = 0, 1, 2, 3, 4
_ATBD = 5  # 4 blocks: 2*c + cp
_ATBQ = 9  # 2 blocks: c
_NCBLK = 11


def _build_nc():
    from contextlib import ExitStack

    import concourse.mybir as mybir
    import concourse.tile as tile
    from concourse import bacc
    from concourse.bass import ds

    fp = mybir.dt.float32
    f16 = mybir.dt.float16
    AF = mybir.ActivationFunctionType

    nc = bacc.Bacc(
        "TRN2",
        target_bir_lowering=False,
        debug=False,
        enable_asserts=False,
        num_devices=NCORES,
    )

    xfd_d = nc.dram_tensor("xfd", [128, 2048], f16, kind="ExternalInput")
    cst_d = nc.dram_tensor("cst", [128, _NCBLK * 128], f16, kind="ExternalInput")
    bias_d = nc.dram_tensor("bias", [128, 2], fp, kind="ExternalInput")
    outfd_d = nc.dram_tensor("outfd", [128, 512], fp, kind="ExternalOutput")

    C512 = [slice(512 * j, 512 * (j + 1)) for j in range(4)]
    H1024 = [slice(1024 * j, 1024 * (j + 1)) for j in range(2)]

    with tile.TileContext(nc) as tc, ExitStack() as ctx:
        const = ctx.enter_context(tc.tile_pool(name="const", bufs=1))
        st = ctx.enter_context(tc.tile_pool(name="st", bufs=1))
        ps = ctx.enter_context(tc.tile_pool(name="ps", bufs=2, space="PSUM"))

        pid = nc.vector.partition_id()  # on DVE: consumed by the z1q copy AP
        cq = (pid % NQ) // 2  # which t-half holds this core's quarter
        tl0 = (pid % 2) * TQ  # tl offset inside that half

        # ---- PE warmup scratch (vector memset: fast, DVE idle this early) ----
        warm_sb = st.tile([128, 512], f16, tag="warm_sb")
        nc.vector.memset(warm_sb[:], 0.0)
        # Preload the tanh activation table off the critical path: without
        # this the first real ACTIVATE pays a lazy 1.3us ACT_TABLE_LOAD.
        tblw = st.tile([128, 1], fp, tag="tblw")
        nc.scalar.activation(tblw[:], warm_sb[:, 0:1], AF.Tanh)

        # ---- loads, chunked so w0pre's first matmuls start before the full
        # transfers land: W1'/P/Q first, then x in halves, then the rest ----
        cs = const.tile([128, _NCBLK * 128], f16, tag="cs")
        x_fd = st.tile([128, 2048], f16, tag="x_fd")
        bt = const.tile([128, 2], fp, tag="bt")
        nc.sync.dma_start(bt[:], bias_d.ap())  # tiny; ACT-w0 needs it first
        nc.sync.dma_start(cs[:, 0:384], cst_d.ap()[:, 0:384])
        for j in range(4):
            nc.sync.dma_start(x_fd[:, C512[j]], xfd_d.ap()[:, C512[j]])
        nc.sync.dma_start(cs[:, 384:], cst_d.ap()[:, 384:])

        def blk(i):
            return cs[:, 128 * i : 128 * (i + 1)]

        w1m, pmat, qmat, h1m, w2m = blk(_W1), blk(_P), blk(_Q), blk(_H1), blk(_W2)
        b1t = bt[:, 0:1]
        b2t = bt[:, 1:2]
        mm = nc.tensor.matmul

        def dummy(out, src):
            """Keep-warm matmul: garbage into a region whose real writer
            opens with start=True. Fires when `src` (SBUF fp16) is ready."""
            mm(out, src[:, 0:128], src[:, 0:512], start=True, stop=True,
               skip_group_check=True)

        # PSUM tiles are split in HALVES (pA/pB): consumers of a PSUM tile
        # wait for its ENTIRE matmul group (no subtile deps on PSUM), so
        # smaller tiles let ACT start at half-group completion.
        # ---- PE warmup: a few matmuls start the HAM clock ramp while the
        # DMAs land. They write u0's buffer, so w0pre has no WAW dependence
        # on them and starts as soon as the DMA semaphores are visible.
        w0a = ps.tile([128, 1024], fp, tag="pA")
        w0b = ps.tile([128, 1024], fp, tag="pB")
        u0a = ps.tile([128, 1024], fp, tag="pA")
        u0b = ps.tile([128, 1024], fp, tag="pB")
        for _ in range(3):
            dummy(u0a[:, 0:512], warm_sb)

        # =========================== layer 1 (full) ===========================
        # w0 = x @ W1' + b1'   -> FD [h-part, (c, tl, nl)]
        w0h = [w0a, w0a, w0b, w0b]
        for j in range(4):
            mm(w0h[j][:, 512 * (j % 2) : 512 * (j % 2 + 1)], w1m, x_fd[:, C512[j]],
               start=True, stop=True)
        w0_fd = st.tile([128, 2048], f16, tag="w0_fd")
        for j in range(4):
            nc.scalar.activation(
                w0_fd[:, C512[j]], w0h[j][:, 512 * (j % 2) : 512 * (j % 2 + 1)],
                AF.Identity, bias=b1t)

        # g0 = FDT of w0 [tl-part, (c, nl, h)]: x ships host-marshalled with
        # free order (c, nl, tl), so this critical-path transpose is
        # contiguous BOTH sides (the strided read moved to off-path w0_nm)
        g0 = st.tile([128, 2048], f16, tag="g0")
        dummy(u0a[:, 0:512], w0_fd)  # fires mid ACT/transpose phase
        for c in range(2):
            nc.vector.transpose(out=g0[:, H1024[c]], in_=w0_fd[:, H1024[c]])

        # u0 = At-mix(w0): FDT, PSUM-accum over c -> free (cp, nl, h).
        # Moving operands are plain contiguous 512-slices of g0 (nl-halves):
        # strided/3D moving APs measured 4 cycles/col vs 1 contiguous.
        # c outermost: all 4 start-matmuls depend only on g0's first t-half,
        # so they overlap the transpose of the second half.
        dummy(u0a[:, 0:512], g0)
        u0h = [u0a, u0b]
        for c in range(2):
            for cp in range(2):
                for nn in range(2):
                    mm(
                        u0h[cp][:, 512 * nn : 512 * (nn + 1)],
                        blk(_ATBD + 2 * c + cp),
                        g0[:, c * 1024 + 512 * nn : c * 1024 + 512 * (nn + 1)],
                        start=(c == 0),
                        stop=(c == 1),
                    )
        u0_sb = st.tile([128, 2048], f16, tag="u0_sb")
        for j in range(2):
            nc.scalar.activation(u0_sb[:, H1024[j]], u0h[j][:], AF.Identity)

        # u0_nm [n, (cp, h, tl')]: strided read, contiguous write, 4-way
        # chunked (h-halves) so each chunk feeds its Q-matmul immediately
        u0_nm = st.tile([128, 2048], f16, tag="u0_nm")
        uiv = u0_sb[:].rearrange("p (cp nl h) -> p cp h nl", cp=2, nl=32, h=32)
        for cp in range(2):
            for hh in range(2):
                nc.vector.transpose(
                    out=u0_nm[:, cp * 1024 + 512 * hh : cp * 1024 + 512 * (hh + 1)],
                    in_=uiv[:, cp, 16 * hh : 16 * (hh + 1), :],
                )

        # w0_nm [n, (c, tl, h)]: strided read (the penalty lands here, off the
        # critical path - w0_nm only feeds P-matmuls that hide under u0)
        w0_nm = st.tile([128, 2048], f16, tag="w0_nm")
        wiv = w0_fd[:].rearrange("p (c nl tl) -> p c tl nl", c=2, nl=32, tl=32)
        for j in range(2):
            nc.vector.transpose(out=w0_nm[:, H1024[j]], in_=wiv[:, j])

        # z1 = tanh(P w0 + Q u0)  -> NM' [n, (c, h, tl)]
        # P moving: w0_nm viewed (c, h, tl) = strided-inner; Q moving: contiguous
        za = ps.tile([128, 1024], fp, tag="pA")
        zb = ps.tile([128, 1024], fp, tag="pB")
        zh = [za, za, zb, zb]
        dummy(za[:, 0:512], u0_sb)
        w0v = w0_nm[:].rearrange("p (c tl h) -> p c h tl", c=2, tl=32, h=32)
        # P/Q interleaved per chunk: the strided P-matmuls (4 cyc/col) would
        # otherwise queue as one block in front of all the Q-matmuls
        for c in range(2):
            for hh in range(2):
                j = 2 * c + hh
                reg = zh[j][:, 512 * (j % 2) : 512 * (j % 2 + 1)]
                mm(reg, pmat, w0v[:, c, 16 * hh : 16 * (hh + 1), :],
                   start=True, stop=False)
                mm(reg, qmat, u0_nm[:, C512[j]], start=False, stop=True)
        z1_nm = st.tile([128, 2048], f16, tag="z1_nm")
        for j in range(4):
            nc.scalar.activation(
                z1_nm[:, C512[j]], zh[j][:, 512 * (j % 2) : 512 * (j % 2 + 1)], AF.Tanh)

        # ====================== layer 2 (t-quarter only) ======================
        # g1 = FDT of z1 [tl-part, (c, h, nl)]: contiguous both sides
        g1 = st.tile([128, 2048], f16, tag="g1")
        u1_ps = ps.tile([128, 1024], fp, tag="pA")
        dummy(u1_ps[:, 0:512], z1_nm)
        for j in range(4):
            nc.vector.transpose(out=g1[:, C512[j]], in_=z1_nm[:, C512[j]])

        # The per-core quarter slice of z1 uses register-offset APs, whose
        # conservative deps (all tanh chunks) would stall the PE semaphore
        # chain - so the dynamic slice is done by a DVE copy into a static
        # tile, and the P-matmul below reads it with a plain contiguous AP.
        vpre = ps.tile([128, 512], fp, tag="pB")
        z1q = st.tile([128, 512], f16, tag="z1q")
        z1v = z1_nm[:].rearrange("p (c h tl) -> p c tl h", c=2, h=32, tl=32)
        nc.vector.tensor_copy(z1q[:], z1v[:, ds(cq, 1), ds(tl0, TQ), :])

        # u1 = At[quarter,:]-mix(z1): PSUM-accum over c -> free (h, nl), part (nh, tq)
        g1r = g1[:].rearrange("p (c h nl) -> p c h nl", c=2, h=32, nl=32)
        for c in range(2):
            for hh in range(2):
                mm(
                    u1_ps[:, 512 * hh : 512 * (hh + 1)],
                    blk(_ATBQ + c),
                    g1r[:, c, 16 * hh : 16 * (hh + 1), :],
                    start=(c == 0),
                    stop=(c == 1),
                )
        u1_sb = st.tile([128, 1024], f16, tag="u1_sb")
        nc.scalar.activation(u1_sb[:], u1_ps[:], AF.Identity)

        # u1_nm [n, (h, tq32)]: contiguous both sides
        u1_nm = st.tile([128, 1024], f16, tag="u1_nm")
        nc.vector.transpose(out=u1_nm[:], in_=u1_sb[:])

        # v = P z1[quarter] + Q u1  -> NM quarter, free (tq, h)
        u1v = u1_nm[:].rearrange("p (h t) -> p t h", h=32, t=32)
        mm(vpre[:], pmat, z1q[:], start=True, stop=False)
        mm(vpre[:], qmat, u1v[:, 0:TQ, :], start=False, stop=True)

        # tail: everything 2-way chunked (tq halves) so the six remaining
        # serial 512-wide ops overlap pairwise across ACT/DVE/PE
        Q256 = [slice(256 * j, 256 * (j + 1)) for j in range(2)]
        v_sb = st.tile([128, 512], f16, tag="v_sb")
        for j in range(2):
            nc.scalar.activation(v_sb[:, Q256[j]], vpre[:, Q256[j]], AF.Identity)

        # v_fd [h-part, (tq, nl)]: contiguous both sides
        v_fd = st.tile([128, 512], f16, tag="v_fd")
        h2pre = ps.tile([128, 512], fp, tag="pA")
        dummy(h2pre[:], v_sb)
        for j in range(2):
            nc.vector.transpose(out=v_fd[:, Q256[j]], in_=v_sb[:, Q256[j]])

        # z2 = tanh(v @ H1')  (FD); out = z2 @ W2' + b2  (FD)
        z2_fd = st.tile([128, 512], f16, tag="z2_fd")
        for j in range(2):
            mm(h2pre[:, Q256[j]], h1m, v_fd[:, Q256[j]], start=True, stop=True)
        for j in range(2):
            nc.scalar.activation(z2_fd[:, Q256[j]], h2pre[:, Q256[j]], AF.Tanh)

        opre = ps.tile([128, 512], fp, tag="pB")
        out_fd = st.tile([128, 512], fp, tag="out_fd")
        for j in range(2):
            mm(opre[:, Q256[j]], w2m, z2_fd[:, Q256[j]], start=True, stop=True)
        for j in range(2):
            nc.scalar.activation(out_fd[:, Q256[j]], opre[:, Q256[j]], AF.Identity, bias=b2t)

        # store in FD layout; the host unscrambles
        nc.sync.dma_start(outfd_d.ap(), out_fd[:])

    nc.compile()
    return nc


def _host_weights(Adj_t, Adj_s, s, H, W1, b1, W2, b2):
    f4 = np.float32
    I4 = np.eye(4, dtype=f4)
    I128 = np.eye(128, dtype=f4)
    Heff = H.sum(axis=1).astype(f4)  # [2, 32, 32]

    P = (s[0] * I128 + s[1] * Adj_s).astype(f4)
    Q = (s[2] * I128 + s[3] * Adj_s).astype(f4)

    W1p = (W1 @ Heff[0]).astype(f4)  # H-first: fold Heff0 into W1
    b1p = (b1 @ Heff[0]).astype(f4)
    w2pad = np.zeros((32, 32), dtype=f4)
    w2pad[:, :FOUT] = W2

    cst = np.zeros((NQ, 128, _NCBLK * 128), dtype=np.float16)
    for q in range(NQ):
        c = cst[q]
        c[:, 0:128] = np.kron(I4, W1p)
        c[:, 128:256] = P
        c[:, 256:384] = Q
        c[:, 384:512] = np.kron(I4, Heff[1])
        c[:, 512:640] = np.kron(I4, w2pad)
        for cc in range(2):
            for cp in range(2):
                i = _ATBD + 2 * cc + cp
                c[:, 128 * i : 128 * (i + 1)] = np.kron(
                    I4, Adj_t[32 * cc : 32 * (cc + 1), 32 * cp : 32 * (cp + 1)].astype(f4)
                )
        for cc in range(2):
            bq = np.zeros((32, 32), dtype=f4)
            bq[:, :TQ] = Adj_t[32 * cc : 32 * (cc + 1), TQ * q : TQ * (q + 1)]
            i = _ATBQ + cc
            c[:, 128 * i : 128 * (i + 1)] = np.kron(I4, bq)

    bias = np.zeros((128, 2), dtype=f4)
    bias[:, 0] = np.tile(b1p, 4)
    b2pad = np.zeros(32, dtype=f4)
    b2pad[:FOUT] = b2
    bias[:, 1] = np.tile(b2pad, 4)
    return cst, bias


def _in_maps(inputs):
    f4 = np.float32
    x = np.asarray(inputs["x"], dtype=f4)
    cst, bias = _host_weights(
        np.asarray(inputs["Adj_t"], dtype=f4),
        np.asarray(inputs["Adj_s"], dtype=f4),
        np.asarray(inputs["s"], dtype=f4),
        np.asarray(inputs["H"], dtype=f4),
        np.asarray(inputs["W1"], dtype=f4),
        np.asarray(inputs["b1"], dtype=f4),
        np.asarray(inputs["W2"], dtype=f4),
        np.asarray(inputs["b2"], dtype=f4),
    )
    # FD-marshalled x per batch, free order (c, nl, tl):
    # xfd[32*nh + f, c*1024 + nl*32 + tl] = x[b, (32*c + tl)*128 + 32*nh + nl, f]
    xfd = [
        np.ascontiguousarray(
            x[b].reshape(2, 32, 4, 32, FIN).transpose(2, 4, 0, 3, 1).reshape(128, 2048)
        ).astype(np.float16)
        for b in range(B)
    ]
    maps = []
    for c in range(NCORES):
        b, q = c // NQ, c % NQ
        maps.append(
            {"xfd": xfd[b], "cst": np.ascontiguousarray(cst[q]), "bias": bias}
        )
    return maps


def kernel(**inputs) -> np.ndarray:
    import os

    from concourse import bass_utils

    if "nc" not in _CACHE:
        _CACHE["nc"] = _build_nc()
    nc = _CACHE["nc"]

    maps = _in_maps(inputs)

    trace = bool(int(os.environ.get("GTCNN_TRACE", "0")))
    res = bass_utils.run_bass_kernel_spmd(
        nc,
        maps,
        core_ids=list(range(NCORES)),
        trace=trace,
        trace_cores=list(range(NCORES)) if trace else None,
        stitch_traces=False,
    )
    _CACHE["last_results"] = res

    out = np.empty((B, M, FOUT), dtype=np.float32)
    for c in range(NCORES):
        b, q = c // NQ, c % NQ
        arr = np.asarray(res.results[c]["outfd"]).reshape(4, 32, TQ, 32)
        out[b, 2048 * q : 2048 * (q + 1), :] = (
            arr[:, :FOUT, :, :].transpose(2, 0, 3, 1).reshape(2048, FOUT)
        )
    return out
